# revision 6
# baseline (speedup 1.0000x reference)
"""Adaptive vector quantizer (progressive VQ codebook) on 8 TRN2 NeuronCores.

Data-parallel: the N=16384 flat rows are sharded 2048/core; the codebook,
per-level swap tables (threefry-derived, input-independent) and column norms
are replicated. Device computes, per core:
  - scores 2s = (2*flat) @ codebook.T  (single fp32 matmul, all 10 levels
    share prefixes of it)
  - v = 2s - fp32(||x||^2 + ||c||^2)   (exactly mirrors the reference's fp32
    rounding of d2 so the per-level argmin tie-breaks match bit-for-bit)
  - per level: first-index argmax over the prefix v[:, :2^(na+1)] (DVE
    max/max_index), then an indirect-DMA row gather from the level's
    swap-aggregated codebook table, and a dense DMA of the gathered rows to
    the output.
Losses are tiny scalar reductions; they are finished on the host from the
device-produced quantized tensors (the "mean all-reduce" of the sharding
hint, done at negligible size), as is the prox term (p x d, host-exact).
"""

import numpy as np

B, T, D, P = 16, 1024, 512, 1024
N = B * T
N_CORES = 8
N_LOC = N // N_CORES          # rows per core
RT = N_LOC // 128             # row-tiles per core
KC = D // 128                 # contraction chunks
N_LEVELS = 10
LAMBDA_C, LAMBDA_P = 0.1, 0.33
NEG_INF = -3.0e38

# ----------------------------------------------------------------------------
# numpy threefry (bit-exact with jax.random's partitionable threefry path)
# ----------------------------------------------------------------------------

def _rotl(x, d):
    return (x << np.uint32(d)) | (x >> np.uint32(32 - d))


def _threefry2x32(k0, k1, x0, x1):
    k0 = np.uint32(k0); k1 = np.uint32(k1)
    ks = [k0, k1, k0 ^ k1 ^ np.uint32(0x1BD11BDA)]
    rot = [[13, 15, 26, 6], [17, 29, 16, 24]]
    x0 = (x0 + ks[0]).astype(np.uint32)
    x1 = (x1 + ks[1]).astype(np.uint32)
    with np.errstate(over="ignore"):
        for i in range(5):
            for r in rot[i % 2]:
                x0 = (x0 + x1).astype(np.uint32)
                x1 = _rotl(x1, r) ^ x0
            x0 = (x0 + ks[(i + 1) % 3]).astype(np.uint32)
            x1 = (x1 + ks[(i + 2) % 3] + np.uint32(i + 1)).astype(np.uint32)
    return x0, x1


def _fold_in(key, data):
    x0, x1 = _threefry2x32(
        key[0], key[1],
        np.asarray([0], np.uint32), np.asarray([data], np.uint32))
    return (x0[0], x1[0])


def _uniform01(key, n):
    c1 = np.zeros(n, dtype=np.uint32)
    c2 = np.arange(n, dtype=np.uint32)
    b1, b2 = _threefry2x32(key[0], key[1], c1, c2)
    fb = ((b1 ^ b2) >> np.uint32(9)) | np.uint32(0x3F800000)
    return fb.view(np.float32) - np.float32(1.0)


def _swap_agg_tables(codebook, correct_p):
    """agg[na][j] = sum_{i: src_na[i] == j} codebook[i] (reference's noisy
    channel column swap, key = fold_in(key(42), na))."""
    base = (np.uint32(0), np.uint32(42))
    cols = np.arange(P, dtype=np.int32)
    tables = []
    for na in range(N_LEVELS):
        u = _uniform01(_fold_in(base, na), P)
        offset = np.floor((u - correct_p) / ((1.0 - correct_p) / P)).astype(np.int32)
        src = np.where(u <= correct_p, cols, (cols + 1 + offset) % P)
        agg = np.zeros_like(codebook)
        np.add.at(agg, src, codebook)
        tables.append(agg)
    return tables


# ----------------------------------------------------------------------------
# device kernel
# ----------------------------------------------------------------------------

def _build_kernel():
    from contextlib import ExitStack
    import concourse.bass as bass
    import concourse.tile as tile
    from concourse import bacc, mybir

    f32 = mybir.dt.float32
    u32 = mybir.dt.uint32

    nc = bacc.Bacc(
        "TRN2", target_bir_lowering=False, debug=False, num_devices=N_CORES
    )

    x2t_d = nc.dram_tensor("x2t", [D, N_LOC], f32, kind="ExternalInput").ap()
    xnt_d = nc.dram_tensor("xnt", [128, RT], f32, kind="ExternalInput").ap()
    cbt_d = nc.dram_tensor("cbt", [D, P], f32, kind="ExternalInput").ap()
    cn_d = nc.dram_tensor("cn128", [128, P], f32, kind="ExternalInput").ap()
    agg_d = [
        nc.dram_tensor(f"agg{na}", [P, D], f32, kind="ExternalInput").ap()
        for na in range(N_LEVELS)
    ]
    out_d = nc.dram_tensor(
        "out", [N_LEVELS, N_LOC, D], f32, kind="ExternalOutput"
    ).ap()

    with tile.TileContext(nc) as tc, ExitStack() as ctx:
        const_p = ctx.enter_context(tc.tile_pool(name="const", bufs=1))
        psum_p = ctx.enter_context(tc.tile_pool(name="psum", bufs=2, space="PSUM"))
        t_p = ctx.enter_context(tc.tile_pool(name="tt", bufs=2))
        v_p = ctx.enter_context(tc.tile_pool(name="vv", bufs=2))
        s_p = ctx.enter_context(tc.tile_pool(name="small", bufs=4))
        q_p = ctx.enter_context(tc.tile_pool(name="qq", bufs=6))

        x2t_sb = [const_p.tile([128, N_LOC], f32, name=f"x2t{k}", tag=f"x2t{k}") for k in range(KC)]
        cbt_sb = [const_p.tile([128, P], f32, name=f"cbt{k}", tag=f"cbt{k}") for k in range(KC)]
        cn_sb = const_p.tile([128, P], f32, name="cn_sb", tag="cn")
        xn_sb = const_p.tile([128, RT], f32, name="xn_sb", tag="xn")

        for k in range(KC):
            nc.sync.dma_start(out=x2t_sb[k][:], in_=x2t_d[k * 128:(k + 1) * 128, :])
            nc.sync.dma_start(out=cbt_sb[k][:], in_=cbt_d[k * 128:(k + 1) * 128, :])
        nc.sync.dma_start(out=cn_sb[:], in_=cn_d[:, :])
        nc.sync.dma_start(out=xn_sb[:], in_=xnt_d[:, :])

        for r in range(RT):
            rs = slice(r * 128, (r + 1) * 128)
            ps = [psum_p.tile([128, 512], f32, name=f"ps{h}", tag=f"ps{h}") for h in range(2)]
            for h in range(2):
                hs = slice(h * 512, (h + 1) * 512)
                for k in range(KC):
                    nc.tensor.matmul(
                        out=ps[h][:],
                        lhsT=x2t_sb[k][:, rs],
                        rhs=cbt_sb[k][:, hs],
                        start=(k == 0),
                        stop=(k == KC - 1),
                    )
            # t = fp32(||x||^2 + ||c||^2) with the reference's rounding order
            t = t_p.tile([128, P], f32, name="t", tag="t")
            nc.vector.tensor_scalar_add(t[:], cn_sb[:], xn_sb[:, r:r + 1])
            # v = 2s - t  (= -d2, single fp32 rounding)
            v = v_p.tile([128, P], f32, name="v", tag="v")
            for h in range(2):
                hs = slice(h * 512, (h + 1) * 512)
                nc.vector.tensor_tensor(
                    out=v[:, hs], in0=ps[h][:], in1=t[:, hs],
                    op=mybir.AluOpType.subtract,
                )
            for na in range(N_LEVELS):
                m = 2 ** (na + 1)
                if m < 8:
                    t8 = s_p.tile([128, 8], f32, name="t8", tag="t8")
                    nc.vector.tensor_copy(t8[:], v[:, :8])
                    nc.vector.memset(t8[:, m:8], NEG_INF)
                    cand = t8[:, :8]
                else:
                    cand = v[:, :m]
                mx = s_p.tile([128, 8], f32, name="mx", tag="mx")
                ix = s_p.tile([128, 8], u32, name="ix", tag="ix")
                nc.vector.max(mx[:], cand)
                nc.vector.max_index(ix[:], mx[:], cand)
                q = q_p.tile([128, D], f32, name="q", tag="q")
                nc.gpsimd.indirect_dma_start(
                    out=q[:],
                    out_offset=None,
                    in_=agg_d[na][:, :],
                    in_offset=bass.IndirectOffsetOnAxis(ap=ix[:, :1], axis=0),
                )
                nc.sync.dma_start(out=out_d[na, rs, :], in_=q[:])

    nc.compile()
    return nc


_NC_CACHE = {}


def _get_nc():
    if "nc" not in _NC_CACHE:
        _NC_CACHE["nc"] = _build_kernel()
    return _NC_CACHE["nc"]


# ----------------------------------------------------------------------------
# host orchestration
# ----------------------------------------------------------------------------

LAST_EXEC_TIME_NS = None


def kernel(inputs, codebook, prev_vecs, correct_p, num_vectors):
    import os
    from concourse.bass_utils import run_bass_kernel_spmd

    inputs = np.asarray(inputs, dtype=np.float32)
    codebook = np.asarray(codebook, dtype=np.float32)
    prev_vecs = np.asarray(prev_vecs, dtype=np.float32)
    correct_p = np.float32(correct_p)
    assert int(num_vectors) == P

    flat = inputs.reshape(N, D)
    xn = np.sum(flat * flat, axis=1)                      # fp32, mirrors jnp
    cn = np.sum(codebook * codebook, axis=1)              # fp32
    agg = _swap_agg_tables(codebook, float(correct_p))

    cbt = np.ascontiguousarray(codebook.T)                # [D, P]
    cn128 = np.ascontiguousarray(np.broadcast_to(cn, (128, P)))

    in_maps = []
    for c in range(N_CORES):
        rows = slice(c * N_LOC, (c + 1) * N_LOC)
        x2t = np.ascontiguousarray((2.0 * flat[rows]).T)  # [D, N_LOC], exact
        xnt = np.ascontiguousarray(xn[rows].reshape(RT, 128).T)  # [128, RT]
        m = {"x2t": x2t, "xnt": xnt, "cbt": cbt, "cn128": cn128}
        for na in range(N_LEVELS):
            m[f"agg{na}"] = agg[na]
        in_maps.append(m)

    nc = _get_nc()
    trace = os.environ.get("VQ_TRACE", "0") == "1"
    res = run_bass_kernel_spmd(
        nc, in_maps, core_ids=list(range(N_CORES)), trace=trace
    )
    global LAST_EXEC_TIME_NS
    LAST_EXEC_TIME_NS = res.exec_time_ns

    shards = [res.results[c]["out"] for c in range(N_CORES)]  # [NL, N_LOC, D]
    quant_flat = np.concatenate(shards, axis=1)               # [NL, N, D]
    quant = quant_flat.reshape(N_LEVELS, B, T, D)

    # losses: scalar means, finished host-side in fp64 from the device output
    losses = np.empty(N_LEVELS, dtype=np.float32)
    flat64 = flat.astype(np.float64)
    for na in range(N_LEVELS):
        dq = quant_flat[na].astype(np.float64) - flat64
        mse = np.mean(dq * dq)
        half = 2 ** (na + 1) // 2
        dp = (prev_vecs[:half].astype(np.float64)
              - codebook[:half].astype(np.float64))
        prox_mse = np.mean(dp * dp)
        if na == 0:
            loss = (1.0 + LAMBDA_C) * mse
        elif na == 1:
            loss = (1.0 + LAMBDA_C) * mse + na * LAMBDA_P * prox_mse
        else:
            loss = mse + LAMBDA_P * prox_mse
        losses[na] = np.float32(loss)

    return quant, losses, codebook.copy()
